# revision 1
# baseline (speedup 1.0000x reference)
"""BarrierNet (MLP heads + dCBF closed-form QP) Trainium2 Bass kernel.

Data-parallel over 8 NeuronCores: batch 262144 is split into 8 shards of
32768 rows; the tiny MLP weights are replicated (folded with mean/std on
host) and each core computes its full shard independently. No collectives.

Per-core dataflow (feature-major matmuls, batch in the free dim), v2:
  Each half (16384 rows) is processed as 8 rounds of 4 chunks (512 rows
  each).  L1 runs 4-way row-group packed (K=8 at PE row offsets 0/32/64/96
  into one [128, 2048] PSUM tile = 4 banks) so all four chunks' h cost a
  single 512-column PE occupancy.  L2 (mid) and L3 (the 4-head
  sliding-window accumulation into one PSUM bank per half) stream
  back-to-back K=128 bf16 matmuls (bf16 keeps the PE HAM clock releasing
  to 2.4 GHz and enables FWL weight loads; measured scaled err ~3e-3);
  the L3 of each round's last chunk is deferred into the next round so
  its relu-copy is never on the PE critical path, and the next round's L1
  + h relu copies are emitted mid-round for the same reason.
  PSUM->SBUF relu copies split Scalar (h chunks 0-2 as one 1536-col op,
  m chunk 1) / Vector (h chunk 3, m chunks 0/2/3).  The dCBF x-side math
  runs once for both halves on [128, 256] batch-major tiles (row =
  half*HB + p*128 + c), emitted in five small stages across half-0's
  rounds (and the staged half-0 QP tail in three pieces across half-1's
  rounds) so neither copy engine ever spikes above the PE's pace;
  sin/cos come from half/quarter-angle identities (sin t =
  2 sin(t/2) cos(t/2), cos(t/2) = 1 - 2 sin^2(t/4)) so every Sin argument
  stays inside the HW table's valid [-pi, pi] range with no wrap ops, and
  the squares/cos reconstructions run as Scalar activations.
  Each half's head-dependent QP tail reshapes the head PSUM layout to
  batch-major via split SBUF->SBUF DMAs; half 0's tail is staged across
  half 1's rounds, half 1's runs at the end with the u store split over
  queues.  A K=128 warmup burst wakes the PE HAM clock during input DMA.
"""

import os
import sys

import numpy as np

sys.path.insert(0, "/opt/trn_rl_repo")

import concourse.bass as bass
import concourse.tile as tile
from concourse import mybir
from concourse.bass_utils import run_bass_kernel_spmd

F32 = mybir.dt.float32
F32R = mybir.dt.float32r
BF16 = mybir.dt.bfloat16
AF = mybir.ActivationFunctionType
ALU = mybir.AluOpType

B = 262144
NF = 8
NCORES = 8
BC = B // NCORES   # 32768 rows per core
CH = 512           # chunk columns (one PSUM bank of fp32)
HB = BC // 2       # 16384 rows per half
NSUB = 4           # L1 row groups (tile_position packing) per half
SUBSZ = HB // NSUB # 4096 rows per subgroup
NRND = SUBSZ // CH # 8 rounds per half
HQ = HB // 128     # 128: per-half batch-major free width
OBS_X, OBS_Y, RAD = 4.0, 6.0, 1.5
PI = float(np.pi)

N_WARMUP_MM = 14

_MMDT_MAP = {"f32r": F32R, "f32": F32, "bf16": BF16}
MMDT = _MMDT_MAP[os.environ.get("KERNEL_MM_DTYPE", "bf16")]
MMNP = {"f32r": np.float32, "f32": np.float32}.get(
    os.environ.get("KERNEL_MM_DTYPE", "bf16"))  # None -> ml_dtypes.bfloat16


def _build_program(std4, mean4, split_waits=True, reps=1):
    nc = bass.Bass()

    x_bm = nc.dram_tensor("x_bm", [BC, NF], F32, kind="ExternalInput")
    x_tr = nc.dram_tensor("x_tr", [8 * NSUB * 2, SUBSZ], MMDT,
                          kind="ExternalInput")
    w1t = nc.dram_tensor("w1t", [8, 128], MMDT, kind="ExternalInput")
    wmw = nc.dram_tensor("wmw", [128, 288], MMDT, kind="ExternalInput")
    bias3 = nc.dram_tensor("bias3", [128, 3], F32, kind="ExternalInput")
    u_out = nc.dram_tensor("u", [BC, 2], F32, kind="ExternalOutput")

    with tile.TileContext(nc) as tc:
        from contextlib import ExitStack

        with ExitStack() as ctx:
            _body(ctx, tc, x_bm, x_tr, w1t, wmw, bias3, u_out,
                  std4, mean4, reps)
    if split_waits:
        _split_multi_waits(nc)
    return nc


def _split_multi_waits(nc):
    """walrus (this build) accepts at most one sync-wait per instruction;
    merge same-semaphore waits to their max threshold, then hoist any
    remaining extra waits onto standalone same-engine EventSemaphore ops."""
    for blk in nc.main_func.blocks:
        out = []
        for ins in blk.instructions:
            si = ins.sync_info
            waits = list(si.on_wait) if si is not None else []
            if len(waits) > 1:
                merged = {}
                for w in waits:
                    key = (w.sync_type, w.id)
                    prev = merged.get(key)
                    if (prev is None or (w.wait_value or 0) >
                            (prev.wait_value or 0)):
                        merged[key] = w
                waits = list(merged.values())
                if len(waits) == 1:
                    ins.sync_info = type(si)(on_wait=waits,
                                             on_update=list(si.on_update))
            if len(waits) > 1:
                for k, w in enumerate(waits[:-1]):
                    ev = mybir.InstEventSemaphore(
                        name=f"{ins.name}w{k}", ins=[], outs=[])
                    ev.engine = ins.engine
                    ev.sync_info = type(si)(on_wait=[w], on_update=[])
                    out.append(ev)
                ins.sync_info = type(si)(on_wait=[waits[-1]],
                                         on_update=list(si.on_update))
            out.append(ins)
        blk.instructions = out
    return nc


def _body(ctx, tc, x_bm, x_tr, w1t, wmw, bias3, u_out,
          std4, mean4, reps):
    nc = tc.nc

    const = ctx.enter_context(tc.tile_pool(name="const", bufs=1))
    xtp = ctx.enter_context(tc.tile_pool(name="xtp", bufs=1))
    hp = ctx.enter_context(tc.tile_pool(name="hp", bufs=2))
    mp = ctx.enter_context(tc.tile_pool(name="mp", bufs=4))
    hs = ctx.enter_context(tc.tile_pool(name="hs", bufs=1))
    qp = ctx.enter_context(tc.tile_pool(name="qp", bufs=1))
    # PSUM: h 4-way [128, 2048] = 4 banks; m [128,512] x2 = 2;
    # two head accumulators = 2  -> exactly 8 banks
    ps_h = ctx.enter_context(tc.tile_pool(name="ps_h", bufs=1, space="PSUM"))
    ps_m = ctx.enter_context(tc.tile_pool(name="ps_m", bufs=2, space="PSUM"))
    ps_hd = ctx.enter_context(tc.tile_pool(name="ps_hd", bufs=1, space="PSUM"))

    # ---- constants / weights to SBUF ----
    # wmw's first 128 cols gate the warmup matmuls: load them first, alone.
    wmw_sb = const.tile([128, 288], MMDT)
    nc.sync.dma_start(out=wmw_sb[:, 0:64], in_=wmw[:, 0:64])
    nc.scalar.dma_start(out=wmw_sb[:, 64:128], in_=wmw[:, 64:128])
    nc.gpsimd.dma_start(out=wmw_sb[:, 128:288], in_=wmw[:, 128:288])
    wmt_sb = wmw_sb[:, 0:128]
    wz_sb = wmw_sb[:, 128:288]

    dummy = const.tile([128, CH], MMDT)
    nc.vector.memset(dummy, 1.0)

    w1g_sb = const.tile([128, 128], MMDT)   # W1eff^T in 2 row groups
    for g in range(NSUB):
        eng = nc.scalar if g % 2 == 0 else nc.gpsimd
        eng.dma_start(out=w1g_sb[32 * g:32 * g + 8, :], in_=w1t[:, :])
    bias3_sb = const.tile([128, 3], F32)
    nc.scalar.dma_start(out=bias3_sb, in_=bias3[:, :])
    b1_sb = bias3_sb[:, 0:1]
    bm_sb = bias3_sb[:, 1:2]
    bh_sb = bias3_sb[:, 2:3]

    for _ in range(reps):
        _body_rep(nc, tc, const, xtp, hp, mp, hs, qp, ps_h, ps_m, ps_hd,
                  x_bm, x_tr, u_out, w1g_sb, wmt_sb, wz_sb, b1_sb, bm_sb,
                  bh_sb, std4, mean4, dummy)


def _qp_pre_stages(nc, qp, x_all, std4, mean4):
    """x-only dCBF terms for the whole core shard, batch-major
    [128, 256] tiles (free index = half*128 + c, row = half*HB + p*128 + c).
    Returns (tiles, [stageA, stageB, stageC]) -- the stages are emitted at
    different rounds so the Vector engine never falls behind the PE."""
    s0, s1c, s2c, s3 = std4
    m0, m1c, m2c, m3 = mean4
    ve = nc.vector
    W = 2 * HQ

    tiles = {}

    def t(name):
        if name not in tiles:
            tiles[name] = qp.tile([128, W], F32, name=name, tag=name)
        return tiles[name]

    xs4 = x_all[:].rearrange("p (w f) -> p w f", f=NF)
    X0, X1, X2, X3 = (xs4[:, :, i] for i in range(4))

    cst = qp.tile([128, 3], F32, name="qcst", tag="qcst")

    def stage_a():
        # constants for activation biases (only 0.0/1.0 are pre-registered)
        nc.vector.memset(cst[:, 1:2], m0 - OBS_X)
        nc.vector.memset(cst[:, 2:3], m1c - OBS_Y)
        # trig via half/quarter angles -- every Sin argument stays inside
        # the HW table's valid [-pi, pi] range (|theta| < 2pi holds for
        # N(0,1) inputs), no wrap ops needed:
        #   cos(t/2) = 1 - 2 sin^2(t/4);  sin t = 2 sin(t/2) cos(t/2);
        #   cos t = 1 - 2 sin^2(t/2)
        ST, CT = t("ST"), t("CT")
        if s2c == 1.0 and m2c == 0.0:
            base = X2
        else:
            base = t("thb")
            ve.tensor_scalar(base, X2, s2c, m2c, ALU.mult, ALU.add)
        s2_, s4_, c2_, tm1, tm2 = (t("s2_"), t("s4_"), t("c2_"),
                                   t("tm1"), t("tm2"))
        sq2 = float(np.sqrt(2.0))
        nc.scalar.activation(s4_, base, AF.Sin, scale=0.25)
        nc.scalar.activation(s2_, base, AF.Sin, scale=0.5)
        nc.scalar.activation(tm1, s4_, AF.Square, scale=sq2)  # 2sin^2(t/4)
        nc.scalar.activation(c2_, tm1, AF.Identity,
                             bias=1.0, scale=-1.0)            # cos(t/2)
        ve.scalar_tensor_tensor(ST, s2_, 2.0, c2_, ALU.mult, ALU.mult)
        nc.scalar.activation(tm2, s2_, AF.Square, scale=sq2)  # 2sin^2(t/2)
        nc.scalar.activation(CT, tm2, AF.Identity,
                             bias=1.0, scale=-1.0)            # cos t
        DX, DY = t("DX"), t("DY")
        nc.scalar.activation(DX, X0, AF.Identity, bias=cst[:, 1:2], scale=s0)
        nc.scalar.activation(DY, X1, AF.Identity, bias=cst[:, 2:3], scale=s1c)

    def stage_b1():
        ST, CT, DX, DY = t("ST"), t("CT"), t("DX"), t("DY")
        if s3 == 1.0 and m3 == 0.0:
            V = X3
        else:
            V = t("V")
            ve.tensor_scalar(V, X3, s3, m3, ALU.mult, ALU.add)
        tiles["Vap"] = V
        t1, t2, Aq, t3 = t("t1"), t("t2"), t("Aq"), t("t3")
        ve.tensor_tensor(t1, DX, CT, ALU.mult)
        ve.tensor_tensor(t2, DY, ST, ALU.mult)
        ve.tensor_tensor(Aq, t1, t2, ALU.add)       # A = dx ct + dy st
        ve.tensor_tensor(t3, DX, ST, ALU.mult)

    def stage_b2():
        ST, CT, DX, DY = t("ST"), t("CT"), t("DX"), t("DY")
        Aq, t3, t4, Bq = t("Aq"), t("t3"), t("t4"), t("Bq")
        V = tiles["Vap"]
        ve.tensor_tensor(t4, DY, CT, ALU.mult)
        ve.tensor_tensor(Bq, t3, t4, ALU.subtract)  # B = dx st - dy ct
        VB, VA = t("VB"), t("VA")
        ve.tensor_tensor(VB, V, Bq, ALU.mult)       # G1 = 2 VB
        ve.tensor_tensor(VA, V, Aq, ALU.mult)       # bdot = 2 VA

    def stage_c1():
        Aq, Bq, VB = t("Aq"), t("Bq"), t("VB")
        V = tiles["Vap"]
        B2, A2, V2d, VB2 = t("B2"), t("A2"), t("V2d"), t("VB2")
        nc.scalar.activation(A2, Aq, AF.Square, scale=2.0)   # G2^2 = 4A^2
        nc.scalar.activation(V2d, V, AF.Square,
                             scale=float(np.sqrt(2.0)))      # 2 v^2
        nc.scalar.activation(VB2, VB, AF.Square, scale=2.0)  # G1^2
        nc.scalar.activation(B2, Bq, AF.Square)

    def stage_c2():
        B2, A2, VB2, BARp = t("B2"), t("A2"), t("VB2"), t("BARp")
        # dx^2 + dy^2 == A^2 + B^2 (rotation identity)
        ve.scalar_tensor_tensor(BARp, A2, 0.25, B2, ALU.mult, ALU.add)
        GG, R = t("GG"), t("R")
        ve.scalar_tensor_tensor(GG, VB2, 1e-12, A2, ALU.add, ALU.add)
        ve.reciprocal(R, GG)

    return tiles, [stage_a, stage_b1, stage_b2, stage_c1, stage_c2]


def _qp_post(nc, qp, half, tiles, headsb, u_out, tail):
    """Head-dependent QP tail for one half.  Returns a list of thunks:
    caller invokes them immediately (tail=True) or spread over the next
    rounds (tail=False)."""
    ve = nc.vector
    sl = slice(half * HQ, (half + 1) * HQ)
    Aq, VB, VA = tiles["Aq"][:, sl], tiles["VB"][:, sl], tiles["VA"][:, sl]
    BARp, V2d, R = (tiles["BARp"][:, sl], tiles["V2d"][:, sl],
                    tiles["R"][:, sl])

    def t(name):
        nm = f"{name}_{half}"
        return qp.tile([128, HQ], F32, name=nm, tag=nm)

    # batch-major head tiles: dst partition p=4j+q, free c: row = 128p + c
    p1n, p2n, sg1, sg2 = t("p1n"), t("p2n"), t("sg1"), t("sg2")
    engs = [nc.sync, nc.gpsimd, nc.scalar]
    # sigmoid heads (sg1, sg2) first: they start the compute chain.
    # The final half's DMAs are split in two for lower tail latency.
    npiece = 2 if tail else 1
    rows16 = 32 // npiece
    for k, (v, dst) in enumerate([(2, sg1), (3, sg2), (0, p1n), (1, p2n)]):
        for piece in range(npiece):
            engs[(npiece * k + piece) % 3].dma_start(
                out=dst[4 * rows16 * piece:4 * rows16 * (piece + 1), :],
                in_=headsb[32 * v + rows16 * piece:
                           32 * v + rows16 * (piece + 1),
                           :].rearrange("j (q c) -> j q c", q=4),
            )

    SS, SP, T5p, T4d = t("SS"), t("SP"), t("T5p"), t("T4d")
    T1d, T2d, T3d, zz, q2, NUMn = (t("T1d"), t("T2d"), t("T3d"),
                                   t("zz"), t("q2"), t("NUMn"))
    L0, LAM2 = t("L0"), t("LAM2")
    u_bm = qp.tile([128, 2 * HQ], F32, name=f"u_bm_{half}",
                   tag=f"u_bm_{half}")
    ub3 = u_bm[:].rearrange("p (c v) -> p c v", v=2)
    m1t, m2t = t("m1t"), t("m2t")

    def part1a():
        # ordered so dependent ops sit >=2 slots behind their producers
        # (each back-to-back dependency costs ~260ns of sem latency)
        ve.scalar_tensor_tensor(T1d, VB, 2.0, p1n, ALU.mult, ALU.mult)
        ve.tensor_tensor(SS, sg1, sg2, ALU.add)
        ve.scalar_tensor_tensor(T2d, Aq, 2.0, p2n, ALU.mult, ALU.mult)
        ve.tensor_tensor(SP, sg1, sg2, ALU.mult)
        ve.scalar_tensor_tensor(T4d, SS, 8.0, VA, ALU.mult, ALU.mult)

    def part1b():
        ve.tensor_tensor(T3d, T1d, T2d, ALU.subtract)  # = -Gp
        ve.tensor_tensor(zz, V2d, T4d, ALU.add)
        ve.scalar_tensor_tensor(T5p, BARp, -RAD * RAD, SP,
                                ALU.add, ALU.mult)
        ve.tensor_tensor(q2, T3d, zz, ALU.subtract)
        ve.scalar_tensor_tensor(NUMn, T5p, 16.0, q2,
                                ALU.mult, ALU.subtract)  # = Gp + hcon

    def part2():
        ve.tensor_tensor(L0, NUMn, R, ALU.mult)
        ve.tensor_scalar(LAM2, L0, -2.0, 0.0, ALU.mult, ALU.max)  # 2 lam
        ve.tensor_tensor(m1t, LAM2, VB, ALU.mult)
        ve.tensor_tensor(m2t, LAM2, Aq, ALU.mult)
        ve.tensor_tensor(ub3[:, :, 0], p1n, m1t, ALU.subtract)
        ve.tensor_tensor(ub3[:, :, 1], p2n, m2t, ALU.add)
        # final store, split across the three DMA queues (one piece per
        # queue) to shorten the drain
        bounds = [0, 44, 86, 128] if tail else [0, 64, 128]
        for piece in range(len(bounds) - 1):
            lo, hi = bounds[piece], bounds[piece + 1]
            engs[piece % 3].dma_start(
                out=u_out[half * HB + lo * HQ:
                          half * HB + hi * HQ, :].rearrange(
                    "(p c) v -> p c v", p=hi - lo),
                in_=ub3[lo:hi],
            )

    if tail:
        part1a()
        part1b()
        part2()
        return []
    return [part1a, part1b, part2]


def _body_rep(nc, tc, const, xtp, hp, mp, hs, qp, ps_h, ps_m, ps_hd,
              x_bm, x_tr, u_out, w1g_sb, wmt_sb, wz_sb, b1_sb, bm_sb, bh_sb,
              std4, mean4, dummy):
    # ---- head accumulators; also the PE-warmup dump target ----
    head_ps = [ps_hd.tile([128, CH], F32, name=f"head{h}", tag=f"head{h}")
               for h in range(2)]

    # PE warmup: K=128 full-array matmuls alternating PSUM banks keep the
    # HAM activity window busy while the input DMAs run.
    for w in range(N_WARMUP_MM):
        nc.tensor.matmul(head_ps[w % 2], wmt_sb, dummy,
                         start=True, stop=True)

    # ---- x loads: first slices of half 0 gate the pipeline start ----
    xt_sb = xtp.tile([128, 2 * SUBSZ], MMDT, name="xt_sb", tag="xt_sb")
    engs = [nc.sync, nc.gpsimd, nc.scalar]
    for g in range(NSUB):
        engs[g % 3].dma_start(
            out=xt_sb[32 * g:32 * g + 8, 0:1024],
            in_=x_tr[8 * g:8 * g + 8, 0:1024])
    for g in range(NSUB):
        # rest of half 0 and all of half 1 for this subgroup: the two DRAM
        # row-blocks are strided NSUB*8 rows apart, expressible as one DMA
        engs[g % 3].dma_start(
            out=xt_sb[32 * g:32 * g + 8, 1024:SUBSZ],
            in_=x_tr[8 * g:8 * g + 8, 1024:SUBSZ])
    for g in range(NSUB):
        engs[g % 3].dma_start(
            out=xt_sb[32 * g:32 * g + 8, SUBSZ:2 * SUBSZ],
            in_=x_tr[8 * (NSUB + g):8 * (NSUB + g) + 8, :])
    # batch-major x for the dCBF math: free = (half, c, f),
    # row = half*HB + p*128 + c
    x_all = xtp.tile([128, 2 * HQ * NF], F32, name="x_all", tag="x_all")
    for h in range(2):
        engs[h % 3].dma_start(
            out=x_all[:].rearrange("p (h c f) -> p h c f",
                                   h=2, f=NF)[:, h],
            in_=x_bm[h * HB:(h + 1) * HB, :].rearrange(
                "(p c) f -> p c f", p=128),
        )

    # prime the Scalar activation table during the input-DMA wait: the
    # first Relu otherwise pays a 1.28us ACT_TABLE_LOAD on the first
    # round's critical path (Relu only -- priming Sin too evicts it)
    prime = const.tile([128, 1], F32, name="actprime", tag="actprime")
    nc.scalar.activation(prime, dummy[:, 0:1], AF.Relu)

    qp_tiles, qp_stages = _qp_pre_stages(nc, qp, x_all, std4, mean4)
    post_thunks = []

    def emit_l1_round(half, s):
        """4-way packed L1 for round (half, s) + the Scalar relu copies of
        h PSUM chunks 0-2.  Chunk 3's Vector relu is returned as a thunk so
        the caller can place it after this round's m copies in the Vector
        stream (it is only needed by the next round's last L2)."""
        h_ps = ps_h.tile([128, NSUB * CH], F32, name="h_ps", tag="h_ps")
        for g in range(NSUB):
            nc.tensor.matmul(
                h_ps[:, g * CH:(g + 1) * CH],
                w1g_sb[32 * g:32 * g + 8, :],
                xt_sb[32 * g:32 * g + 8,
                      half * SUBSZ + s * CH:half * SUBSZ + (s + 1) * CH],
                start=True, stop=True,
                tile_position=(32 * g, 0),
            )
        h_sb = hp.tile([128, NSUB * CH], MMDT, name="h_sb", tag="h_sb")
        nc.scalar.activation(h_sb[:, 0:3 * CH], h_ps[:, 0:3 * CH],
                             AF.Relu, bias=b1_sb, scale=1.0)

        def vec_part():
            nc.vector.tensor_scalar(h_sb[:, 3 * CH:4 * CH],
                                    h_ps[:, 3 * CH:4 * CH],
                                    b1_sb, 0.0, ALU.add, ALU.max)
        return h_sb, vec_part

    # ---- MLP round pipeline: half 0 fully, then half 1 ----
    h_sb_cur, vec0 = emit_l1_round(0, 0)
    vec0()
    h_sb_nxt = None
    pend_l3 = []  # deferred [(half, wz window, m_sb, stop)] from prev round

    for half in range(2):
        for s in range(NRND):
            m_sbs = []

            def l2(g):
                m_ps = ps_m.tile([128, CH], F32, name="m_ps", tag="m_ps")
                nc.tensor.matmul(
                    m_ps, wmt_sb, h_sb_cur[:, g * CH:(g + 1) * CH],
                    start=True, stop=True)
                m_sb = mp.tile([128, CH], MMDT, name="m_sb", tag="m_sb")
                if g == 1:
                    nc.scalar.activation(m_sb, m_ps, AF.Relu, bias=bm_sb,
                                         scale=1.0)
                else:
                    nc.vector.tensor_scalar(m_sb, m_ps, bm_sb, 0.0,
                                            ALU.add, ALU.max)
                m_sbs.append(m_sb)

            def l3(g):
                jh = 8 * g + s
                step = 4 * s + g
                nc.tensor.matmul(
                    head_ps[half],
                    wz_sb[:, 31 - jh:159 - jh],
                    m_sbs[g],
                    start=(step == 0), stop=(step == 31),
                )

            l2(0)
            l2(1)
            for ph, pwin, pm, pstop in pend_l3:
                nc.tensor.matmul(head_ps[ph], pwin, pm,
                                 start=False, stop=pstop)
            pend_l3 = []
            # next round's L1 (pipeline): emitted mid-stream so its weight
            # load and the h relu copies overlap this round's L2/L3s
            nxt = (half, s + 1) if s + 1 < NRND else (
                (1, 0) if half == 0 else None)
            vec_part = None
            if nxt is not None:
                h_sb_nxt, vec_part = emit_l1_round(*nxt)
            l3(0)
            l2(2)
            l3(1)
            l2(3)
            l3(2)
            if vec_part is not None:
                vec_part()
            # chunk 3's head matmul is deferred into the next round
            jh3 = 8 * 3 + s
            pend_l3 = [(half, wz_sb[:, 31 - jh3:159 - jh3], m_sbs[3],
                        4 * s + 3 == 31)]

            if half == 0 and s == 0:
                # pull the Sin table-bank load into round 0's Scalar slack
                # (stage_a's first Sin otherwise pays it mid-phase)
                nc.scalar.activation(prime, dummy[:, 0:1], AF.Sin)
            if half == 0 and 2 <= s <= 6:
                qp_stages[s - 2]()
            if half == 1 and s in (1, 2, 4) and post_thunks:
                post_thunks.pop(0)()

            h_sb_cur = h_sb_nxt

        # flush the deferred L3s of this half, then drain heads to QP
        for ph, pwin, pm, pstop in pend_l3:
            nc.tensor.matmul(head_ps[ph], pwin, pm, start=False, stop=pstop)
        pend_l3 = []

        hsb = hs.tile([128, CH], F32, name=f"hsb{half}", tag=f"hsb{half}")
        nc.scalar.activation(hsb[64:128, :], head_ps[half][64:128, :],
                             AF.Sigmoid, bias=bh_sb[64:128, :], scale=1.0)
        if half == 1:
            # tail half: run the p-head drain on Vector (idle here) so the
            # two drains are parallel instead of serialized on Scalar
            nc.vector.tensor_scalar(hsb[0:64, :], head_ps[half][0:64, :],
                                    -1.0, bh_sb[0:64, :],
                                    ALU.mult, ALU.add)
        else:
            nc.scalar.activation(hsb[0:64, :], head_ps[half][0:64, :],
                                 AF.Identity, bias=bh_sb[0:64, :],
                                 scale=-1.0)
        post_thunks = _qp_post(nc, qp, half, qp_tiles, hsb, u_out,
                               tail=(half == 1))


def _host_prepare(inputs):
    """Fold mean/std into L1, build packed weight/bias tensors."""
    x = np.ascontiguousarray(inputs["x"], dtype=np.float32)
    mean = np.asarray(inputs["mean"], dtype=np.float32)
    std = np.asarray(inputs["std"], dtype=np.float32)
    W1 = np.asarray(inputs["W1"], dtype=np.float32)
    b1 = np.asarray(inputs["b1"], dtype=np.float32)
    W21 = np.asarray(inputs["W21"], dtype=np.float32)
    b21 = np.asarray(inputs["b21"], dtype=np.float32)
    W22 = np.asarray(inputs["W22"], dtype=np.float32)
    b22 = np.asarray(inputs["b22"], dtype=np.float32)
    W31 = np.asarray(inputs["W31"], dtype=np.float32)
    b31 = np.asarray(inputs["b31"], dtype=np.float32)
    W32 = np.asarray(inputs["W32"], dtype=np.float32)
    b32 = np.asarray(inputs["b32"], dtype=np.float32)

    W1eff = W1 * std[None, :]                      # [128, 8]
    b1eff = (b1 + W1 @ mean).astype(np.float32)    # [128]
    w1t = np.ascontiguousarray(W1eff.T)            # [8, 128]

    Wmid = np.vstack([W21, W22]).astype(np.float32)   # [128, 128]
    wmt = np.ascontiguousarray(Wmid.T)
    bmid = np.concatenate([b21, b22]).astype(np.float32)[:, None]

    Whead = np.zeros((4, 128), np.float32)
    Whead[0:2, 0:64] = W31
    Whead[2:4, 64:128] = W32
    wz = np.zeros((128, 160), np.float32)
    for v in range(4):
        wz[:, 31 + 32 * v] = Whead[v, :]

    bhead = np.zeros((128, 1), np.float32)
    bhead[0:32, 0] = -b31[0]
    bhead[32:64, 0] = -b31[1]
    bhead[64:96, 0] = b32[0]
    bhead[96:128, 0] = b32[1]

    std4 = tuple(float(std[i]) for i in range(4))
    mean4 = tuple(float(mean[i]) for i in range(4))

    if MMNP is None:
        import ml_dtypes
        mmnp = ml_dtypes.bfloat16
    else:
        mmnp = MMNP
    w1t = w1t.astype(mmnp)
    wmt = wmt.astype(mmnp)
    wz = wz.astype(mmnp)

    wmw = np.ascontiguousarray(np.concatenate([wmt, wz], axis=1))
    bias3 = np.ascontiguousarray(
        np.concatenate([b1eff[:, None], bmid, bhead], axis=1))

    common = {
        "w1t": w1t,
        "wmw": wmw,
        "bias3": bias3,
    }

    in_maps = []
    for c in range(NCORES):
        xs = x[c * BC:(c + 1) * BC]               # [32768, 8]
        # row 8*(NSUB*h+g)+f = feature f of (half h, subgroup g)
        xtr = np.ascontiguousarray(
            xs.reshape(2, NSUB, SUBSZ, NF).transpose(0, 1, 3, 2).reshape(
                8 * NSUB * 2, SUBSZ)).astype(mmnp)
        in_maps.append({"x_bm": xs, "x_tr": xtr, **common})
    return in_maps, std4, mean4


def kernel(**inputs):
    in_maps, std4, mean4 = _host_prepare(inputs)
    nc = _build_program(std4, mean4)
    last_err = None
    for attempt in range(3):
        try:
            res = run_bass_kernel_spmd(nc, in_maps, list(range(NCORES)))
            break
        except Exception as e:  # transient axon/NRT flakes
            last_err = e
            if attempt == 2:
                raise
            import time

            # observed NRT_EXEC_UNIT_UNRECOVERABLE wedges recover with a
            # core reset plus a short pause before the retry
            os.environ["NEURON_RT_RESET_CORES"] = "1"
            time.sleep(15)
    u = np.concatenate([res.results[c]["u"] for c in range(NCORES)], axis=0)
    return u.astype(np.float32)


if __name__ == "__main__":
    rng = np.random.default_rng(0)
    demo = {
        "x": rng.standard_normal((B, NF), dtype=np.float32),
        "mean": np.zeros(NF, np.float32),
        "std": np.ones(NF, np.float32),
        "W1": rng.standard_normal((128, NF), dtype=np.float32) * 0.3,
        "b1": rng.standard_normal(128, dtype=np.float32) * 0.3,
        "W21": rng.standard_normal((64, 128), dtype=np.float32) * 0.08,
        "b21": rng.standard_normal(64, dtype=np.float32) * 0.08,
        "W22": rng.standard_normal((64, 128), dtype=np.float32) * 0.08,
        "b22": rng.standard_normal(64, dtype=np.float32) * 0.08,
        "W31": rng.standard_normal((2, 64), dtype=np.float32) * 0.1,
        "b31": rng.standard_normal(2, dtype=np.float32) * 0.1,
        "W32": rng.standard_normal((2, 64), dtype=np.float32) * 0.1,
        "b32": rng.standard_normal(2, dtype=np.float32) * 0.1,
        "sgn": np.int64(1),
    }
    out = kernel(**demo)
    print(out.shape, out.dtype)

